# revision 21
# baseline (speedup 1.0000x reference)
"""AnchorwCrossEntropyLoss (debiased Sinkhorn anchor loss) — TRN2 Bass kernel.

Math note (why the device kernel is tiny):
The reference computes a debiased Sinkhorn divergence between, per sample b,
a degenerate cloud of M identical copies of logits[b] and the M anchor rows.
Because the x-cloud points are identical per sample:
  * f_aa is identically 0,
  * g_bb only involves anchor-anchor distances -> sample-independent, host-computable,
  * f_ba is a per-sample scalar and (g_ab - dxy) stays constant across anchors,
    which collapses the whole symmetric eps-scaling loop in closed form.
The surviving value is    dist[b] = mean_j ||x_b - a_j||  -  mean_i(g_bb_n[i])
(verified < 1e-7 rel err against the full reference).  The device work is the
masked mean of per-sample anchor-distance means; the tiny (21,) g_bb recursion
and the eps schedule (both O(m^2 * n_iters) ~ 5k flops) run on host, exactly as
the reference itself computes the diameter/eps schedule on host.

Anchors are the fixed set_anchors matrix diag(+5)/off(-5), so
  ||x_b - a_j||^2 = sum_k (x_bk + 5)^2 - 20 * x_bj
which the kernel computes with one Square activation, a segmented reduce, one
fused scalar_tensor_tensor, one Sqrt, and a segmented reduce per tile.
"""

import os
import sys

import numpy as np

for _p in ("/opt/trn_rl_repo",):
    if _p not in sys.path and os.path.isdir(_p):
        sys.path.append(_p)


def _ensure_ntff_hook():
    """The agent image lacks antenv.axon_hooks; shim it so trace=True works."""
    import types
    try:
        import antenv.axon_hooks  # noqa: F401
        return
    except ImportError:
        pass
    try:
        import antenv
        from trn_agent_boot.trn_boot import _ntff_profile_via_ctypes
        mod = types.ModuleType("antenv.axon_hooks")
        _hook = [None]
        mod.set_axon_ntff_profile_hook = lambda h: _hook.__setitem__(0, h)
        mod.get_axon_ntff_profile_hook = lambda: _hook[0]
        sys.modules["antenv.axon_hooks"] = mod
        antenv.axon_hooks = mod
        mod.set_axon_ntff_profile_hook(
            _ntff_profile_via_ctypes("/opt/axon/libaxon_pjrt.so"))
    except Exception:
        pass

NUM_CLASSES = 20
M = NUM_CLASSES + 1          # 21
BLUR = 0.1
SCALING = 0.5
ANCHOR_WEIGHT = 0.1
LOSS_WEIGHT = 1.0
N_ROIS = 32768
N_CORES = 8
N_SH = N_ROIS // N_CORES     # 4096 rois per core
P = 128                      # partitions
R = N_SH // P                # 32 samples per partition

# knobs (env-tunable for experiments)
N_CHUNKS = int(os.environ.get("KERNEL_CHUNKS", "2"))

LAST_EXEC_NS = None
LAST_RESULTS = None

_built = None


def _default_anchors() -> np.ndarray:
    return np.where(np.eye(M, dtype=bool), 5.0, -5.0).astype(np.float32)


def _eps_schedule(diameter: float, blur: float, scaling: float) -> list:
    return ([diameter]
            + [float(np.exp(e))
               for e in np.arange(np.log(diameter), np.log(blur), np.log(scaling))]
            + [blur])


def _host_gbb_mean(cls_score: np.ndarray, anchors: np.ndarray) -> float:
    """mean_i(g_bb_n[i]) of the reference, computed exactly on host (f64)."""
    pts = np.concatenate([np.asarray(cls_score), np.asarray(anchors)], axis=0)
    diameter = float(np.linalg.norm(pts.max(axis=0) - pts.min(axis=0)))
    eps_list = _eps_schedule(diameter, BLUR, SCALING)

    a = np.asarray(anchors, dtype=np.float64)
    A = np.sqrt(((a[:, None, :] - a[None, :, :]) ** 2).sum(-1))  # (M, M)
    bl = -np.log(M)

    def lse(v):  # rowwise logsumexp over last axis
        mx = v.max(axis=-1, keepdims=True)
        return (mx + np.log(np.exp(v - mx).sum(axis=-1, keepdims=True)))[..., 0]

    eps0 = eps_list[0]
    g = -eps0 * lse(bl - A / eps0)
    for eps in eps_list:
        gt = -eps * lse(bl + g[None, :] / eps - A / eps)
        g = 0.5 * (g + gt)
    blur = eps_list[-1]
    g_n = -blur * lse(bl + g[None, :] / blur - A / blur)
    return float(g_n.mean())


def _build_nc():
    import concourse.tile as tile
    from concourse import bacc, mybir

    f32 = mybir.dt.float32
    i32 = mybir.dt.int32
    AF = mybir.ActivationFunctionType
    OP = mybir.AluOpType
    AX = mybir.AxisListType

    assert R % N_CHUNKS == 0
    RC = R // N_CHUNKS           # samples-per-partition per chunk
    W = RC * M                   # floats per partition per chunk

    nc = bacc.Bacc(None, target_bir_lowering=False)
    x_d = nc.declare_dram_parameter("cls_score", [N_SH, M], f32, isOutput=False)
    l_d = nc.declare_dram_parameter("label", [N_SH], i32, isOutput=False)
    out_d = nc.declare_dram_parameter("out", [P, 2], f32, isOutput=True)

    # partition p owns rows [R*p, R*(p+1)) -> contiguous 84*R bytes per partition
    x_v = x_d.rearrange("(p c r) m -> c p (r m)", p=P, c=N_CHUNKS)
    l_v = l_d.rearrange("(p r) -> p r", p=P)

    with tile.TileContext(nc) as tc:
        with (
            tc.tile_pool(name="io", bufs=2) as io_pool,
            tc.tile_pool(name="tmp", bufs=2) as tmp_pool,
            tc.tile_pool(name="acc", bufs=1) as acc_pool,
        ):
            outt = acc_pool.tile([P, 2 * N_CHUNKS], f32)

            # dedicated input tiles per chunk: DMAs never reuse slots, so each
            # DMA carries zero sync waits (HW DMA-direct allows only one).
            xts = [io_pool.tile([P, W], f32, tag=f"xt{c}", name=f"xt{c}")
                   for c in range(N_CHUNKS)]
            lt_all = io_pool.tile([P, R], i32, name="lt_all")
            nc.scalar.dma_start(lt_all[:], l_v)
            for c in range(N_CHUNKS):
                nc.sync.dma_start(xts[c][:], x_v[c])

            for c in range(N_CHUNKS):
                xt = xts[c]
                lt = lt_all[:, c * RC:(c + 1) * RC]

                def T(shape, nm, dt=f32):
                    return tmp_pool.tile(shape, dt, tag=f"{nm}{c}",
                                         name=f"{nm}{c}")

                # t = x^2 + 10x ; base[p,r] = sum_j t + 525 = ||x+5*1||^2
                sq = T([P, W], "sq")
                nc.vector.scalar_tensor_tensor(
                    sq[:], in0=xt[:], scalar=10.0, in1=xt[:],
                    op0=OP.add, op1=OP.mult)
                base_raw = T([P, RC], "base_raw")
                nc.vector.reduce_sum(
                    base_raw[:], sq[:].rearrange("p (r m) -> p r m", m=M), axis=AX.X)
                base = T([P, RC], "base")
                nc.vector.tensor_scalar(base[:], base_raw[:], 525.0, None, OP.add)

                # d2 = -20*x + base  (broadcast base over j); d = sqrt(d2)
                d2 = T([P, W], "d2")
                nc.vector.scalar_tensor_tensor(
                    d2[:].rearrange("p (r m) -> p r m", m=M),
                    in0=xt[:].rearrange("p (r m) -> p r m", m=M),
                    scalar=-20.0,
                    in1=base[:].unsqueeze(2).broadcast_to((P, RC, M)),
                    op0=OP.mult, op1=OP.add)
                d = T([P, W], "d")
                nc.scalar.activation(d[:], d2[:], AF.Sqrt)
                dsum = T([P, RC], "dsum")
                nc.vector.reduce_sum(
                    dsum[:], d[:].rearrange("p (r m) -> p r m", m=M), axis=AX.X)

                # vmask = (label != 20); count (per partition) via accum_out
                labf = T([P, RC], "labf")
                nc.vector.tensor_copy(labf[:], lt)
                vmask = T([P, RC], "vmask")
                nc.vector.tensor_scalar(
                    vmask[:], labf[:], 20.0, None, OP.not_equal, op1=OP.add,
                    accum_out=outt[:, 2 * c + 1:2 * c + 2])
                masked = T([P, RC], "masked")
                nc.vector.scalar_tensor_tensor(
                    masked[:], in0=dsum[:], scalar=1.0, in1=vmask[:],
                    op0=OP.mult, op1=OP.mult,
                    accum_out=outt[:, 2 * c:2 * c + 1])

            # fold chunk partials: out[:, 0] = sum_c dsum_c, out[:, 1] = sum_c cnt_c
            outf = acc_pool.tile([P, 2], f32)
            nc.vector.reduce_sum(
                outf[:], outt[:].rearrange("p (c k) -> p k c", k=2), axis=AX.X)
            nc.scalar.dma_start(out_d[:], outf[:])
    nc.finalize()
    return nc


def _get_built():
    global _built
    if _built is None:
        _built = _build_nc()
    return _built


def kernel(cls_score: np.ndarray, anchors: np.ndarray = None,
           label: np.ndarray = None) -> np.ndarray:
    global LAST_EXEC_NS, LAST_RESULTS
    from concourse.bass_utils import run_bass_kernel_spmd

    cls_score = np.ascontiguousarray(np.asarray(cls_score, dtype=np.float32))
    label = np.ascontiguousarray(np.asarray(label, dtype=np.int32))
    if anchors is None:
        anchors = _default_anchors()
    anchors = np.asarray(anchors, dtype=np.float32)
    assert cls_score.shape == (N_ROIS, M) and label.shape == (N_ROIS,)

    gbb_mean = _host_gbb_mean(cls_score, anchors)

    nc = _get_built()
    in_maps = []
    for i in range(N_CORES):
        sl = slice(i * N_SH, (i + 1) * N_SH)
        in_maps.append({
            "cls_score": np.ascontiguousarray(cls_score[sl]),
            "label": np.ascontiguousarray(label[sl]),
        })

    trace = os.environ.get("KERNEL_TRACE", "0") == "1"
    if trace:
        _ensure_ntff_hook()
    res = run_bass_kernel_spmd(nc, in_maps, core_ids=list(range(N_CORES)),
                               trace=trace)
    LAST_EXEC_NS = res.exec_time_ns
    LAST_RESULTS = res

    outs = np.stack([r["out"] for r in res.results])   # (8, 128, 2)
    d_total = float(outs[:, :, 0].sum(dtype=np.float64))
    n_valid = int(round(float(outs[:, :, 1].sum(dtype=np.float64))))

    loss = (LOSS_WEIGHT * ANCHOR_WEIGHT
            * (d_total / M - gbb_mean * n_valid) / max(n_valid, 1))
    return np.float32(loss)


# revision 23
# speedup vs baseline: 1.0318x; 1.0318x over previous
"""AnchorwCrossEntropyLoss (debiased Sinkhorn anchor loss) — TRN2 Bass kernel.

Math note (why the device kernel is tiny):
The reference computes a debiased Sinkhorn divergence between, per sample b,
a degenerate cloud of M identical copies of logits[b] and the M anchor rows.
Because the x-cloud points are identical per sample:
  * f_aa is identically 0,
  * g_bb only involves anchor-anchor distances -> sample-independent, host-computable,
  * f_ba is a per-sample scalar and (g_ab - dxy) stays constant across anchors,
    which collapses the whole symmetric eps-scaling loop in closed form.
The surviving value is    dist[b] = mean_j ||x_b - a_j||  -  mean_i(g_bb_n[i])
(verified < 1e-7 rel err against the full reference).  The device work is the
masked mean of per-sample anchor-distance means; the tiny (21,) g_bb recursion
and the eps schedule (both O(m^2 * n_iters) ~ 5k flops) run on host, exactly as
the reference itself computes the diameter/eps schedule on host.

Anchors are the fixed set_anchors matrix diag(+5)/off(-5), so
  ||x_b - a_j||^2 = sum_k (x_bk + 5)^2 - 20 * x_bj
which the kernel computes with one Square activation, a segmented reduce, one
fused scalar_tensor_tensor, one Sqrt, and a segmented reduce per tile.
"""

import os
import sys

import numpy as np

for _p in ("/opt/trn_rl_repo",):
    if _p not in sys.path and os.path.isdir(_p):
        sys.path.append(_p)


def _ensure_ntff_hook():
    """The agent image lacks antenv.axon_hooks; shim it so trace=True works."""
    import types
    try:
        import antenv.axon_hooks  # noqa: F401
        return
    except ImportError:
        pass
    try:
        import antenv
        from trn_agent_boot.trn_boot import _ntff_profile_via_ctypes
        mod = types.ModuleType("antenv.axon_hooks")
        _hook = [None]
        mod.set_axon_ntff_profile_hook = lambda h: _hook.__setitem__(0, h)
        mod.get_axon_ntff_profile_hook = lambda: _hook[0]
        sys.modules["antenv.axon_hooks"] = mod
        antenv.axon_hooks = mod
        mod.set_axon_ntff_profile_hook(
            _ntff_profile_via_ctypes("/opt/axon/libaxon_pjrt.so"))
    except Exception:
        pass

NUM_CLASSES = 20
M = NUM_CLASSES + 1          # 21
BLUR = 0.1
SCALING = 0.5
ANCHOR_WEIGHT = 0.1
LOSS_WEIGHT = 1.0
N_ROIS = 32768
N_CORES = 8
N_SH = N_ROIS // N_CORES     # 4096 rois per core
P = 128                      # partitions
R = N_SH // P                # 32 samples per partition

# knobs (env-tunable for experiments)
N_CHUNKS = int(os.environ.get("KERNEL_CHUNKS", "2"))

LAST_EXEC_NS = None
LAST_RESULTS = None

_built = None


def _default_anchors() -> np.ndarray:
    return np.where(np.eye(M, dtype=bool), 5.0, -5.0).astype(np.float32)


def _eps_schedule(diameter: float, blur: float, scaling: float) -> list:
    return ([diameter]
            + [float(np.exp(e))
               for e in np.arange(np.log(diameter), np.log(blur), np.log(scaling))]
            + [blur])


def _host_gbb_mean(cls_score: np.ndarray, anchors: np.ndarray) -> float:
    """mean_i(g_bb_n[i]) of the reference, computed exactly on host (f64)."""
    pts = np.concatenate([np.asarray(cls_score), np.asarray(anchors)], axis=0)
    diameter = float(np.linalg.norm(pts.max(axis=0) - pts.min(axis=0)))
    eps_list = _eps_schedule(diameter, BLUR, SCALING)

    a = np.asarray(anchors, dtype=np.float64)
    A = np.sqrt(((a[:, None, :] - a[None, :, :]) ** 2).sum(-1))  # (M, M)
    bl = -np.log(M)

    def lse(v):  # rowwise logsumexp over last axis
        mx = v.max(axis=-1, keepdims=True)
        return (mx + np.log(np.exp(v - mx).sum(axis=-1, keepdims=True)))[..., 0]

    eps0 = eps_list[0]
    g = -eps0 * lse(bl - A / eps0)
    for eps in eps_list:
        gt = -eps * lse(bl + g[None, :] / eps - A / eps)
        g = 0.5 * (g + gt)
    blur = eps_list[-1]
    g_n = -blur * lse(bl + g[None, :] / blur - A / blur)
    return float(g_n.mean())


def _make_tile_context_cls():
    """TileContext with a lightweight kernel tail.

    Stock Tile ends with drain + all-engine barrier + sem clears + second
    all-engine barrier (~3-5us of EVSEM ping-pong).  All we actually need for
    a correct, re-executable NEFF is: one instruction that waits until every
    tracked semaphore reached its final value, then the gpsimd sem clears
    (same engine -> program order).  Every engine then simply ends; the
    runtime completes the NEFF when all engines halt.
    """
    import concourse.tile as tile
    from concourse.vector_clock import ScopedClock

    class FastEndTileContext(tile.TileContext):
        def _drain_and_barrier(self, tick_clock, wait_clock):
            nc = self.nc
            gate = nc.gpsimd.nop(nofuse=True, hint="tail_gate")
            wait_clock.add_sem_waits(
                gate.ins, ScopedClock({None: tick_clock.global_clock}))
            popped = nc._tile_sem_poison_stack.pop()
            assert popped is self._sem_poison
            nc.clear_and_free_semaphores(list(self.sems.allocated().values()))

    return FastEndTileContext


def _build_nc():
    import concourse.tile as tile
    from concourse import bacc, mybir

    f32 = mybir.dt.float32
    i32 = mybir.dt.int32
    AF = mybir.ActivationFunctionType
    OP = mybir.AluOpType
    AX = mybir.AxisListType

    assert R % N_CHUNKS == 0
    RC = R // N_CHUNKS           # samples-per-partition per chunk
    W = RC * M                   # floats per partition per chunk

    nc = bacc.Bacc(None, target_bir_lowering=False)
    x_d = nc.declare_dram_parameter("cls_score", [N_SH, M], f32, isOutput=False)
    l_d = nc.declare_dram_parameter("label", [N_SH], i32, isOutput=False)
    out_d = nc.declare_dram_parameter("out", [P, 2], f32, isOutput=True)

    # partition p owns rows [R*p, R*(p+1)) -> contiguous 84*R bytes per partition
    x_v = x_d.rearrange("(p c r) m -> c p (r m)", p=P, c=N_CHUNKS)
    l_v = l_d.rearrange("(p r) -> p r", p=P)

    tc_cls = (_make_tile_context_cls()
              if os.environ.get("KERNEL_FAST_END", "1") == "1"
              else tile.TileContext)
    with tc_cls(nc) as tc:
        with (
            tc.tile_pool(name="io", bufs=2) as io_pool,
            tc.tile_pool(name="tmp", bufs=2) as tmp_pool,
            tc.tile_pool(name="acc", bufs=1) as acc_pool,
        ):
            outt = acc_pool.tile([P, 2 * N_CHUNKS], f32)

            # dedicated input tiles per chunk: DMAs never reuse slots, so each
            # DMA carries zero sync waits (HW DMA-direct allows only one).
            xts = [io_pool.tile([P, W], f32, tag=f"xt{c}", name=f"xt{c}")
                   for c in range(N_CHUNKS)]
            lt_all = io_pool.tile([P, R], i32, name="lt_all")
            nc.scalar.dma_start(lt_all[:], l_v)
            for c in range(N_CHUNKS):
                nc.sync.dma_start(xts[c][:], x_v[c])

            for c in range(N_CHUNKS):
                xt = xts[c]
                lt = lt_all[:, c * RC:(c + 1) * RC]

                def T(shape, nm, dt=f32):
                    return tmp_pool.tile(shape, dt, tag=f"{nm}{c}",
                                         name=f"{nm}{c}")

                # t = x^2 + 10x ; base[p,r] = sum_j t + 525 = ||x+5*1||^2
                sq = T([P, W], "sq")
                nc.vector.scalar_tensor_tensor(
                    sq[:], in0=xt[:], scalar=10.0, in1=xt[:],
                    op0=OP.add, op1=OP.mult)
                base_raw = T([P, RC], "base_raw")
                nc.vector.reduce_sum(
                    base_raw[:], sq[:].rearrange("p (r m) -> p r m", m=M), axis=AX.X)
                base = T([P, RC], "base")
                nc.vector.tensor_scalar(base[:], base_raw[:], 525.0, None, OP.add)

                # d2 = -20*x + base  (broadcast base over j); d = sqrt(d2)
                d2 = T([P, W], "d2")
                nc.vector.scalar_tensor_tensor(
                    d2[:].rearrange("p (r m) -> p r m", m=M),
                    in0=xt[:].rearrange("p (r m) -> p r m", m=M),
                    scalar=-20.0,
                    in1=base[:].unsqueeze(2).broadcast_to((P, RC, M)),
                    op0=OP.mult, op1=OP.add)
                d = T([P, W], "d")
                nc.scalar.activation(d[:], d2[:], AF.Sqrt)
                dsum = T([P, RC], "dsum")
                nc.vector.reduce_sum(
                    dsum[:], d[:].rearrange("p (r m) -> p r m", m=M), axis=AX.X)

                # vmask = (label != 20); count (per partition) via accum_out
                labf = T([P, RC], "labf")
                nc.vector.tensor_copy(labf[:], lt)
                vmask = T([P, RC], "vmask")
                nc.vector.tensor_scalar(
                    vmask[:], labf[:], 20.0, None, OP.not_equal, op1=OP.add,
                    accum_out=outt[:, 2 * c + 1:2 * c + 2])
                masked = T([P, RC], "masked")
                nc.vector.scalar_tensor_tensor(
                    masked[:], in0=dsum[:], scalar=1.0, in1=vmask[:],
                    op0=OP.mult, op1=OP.mult,
                    accum_out=outt[:, 2 * c:2 * c + 1])

            # fold chunk partials: out[:, 0] = sum_c dsum_c, out[:, 1] = sum_c cnt_c
            outf = acc_pool.tile([P, 2], f32)
            nc.vector.reduce_sum(
                outf[:], outt[:].rearrange("p (c k) -> p k c", k=2), axis=AX.X)
            nc.scalar.dma_start(out_d[:], outf[:])
    nc.finalize()
    return nc


def _get_built():
    global _built
    if _built is None:
        _built = _build_nc()
    return _built


def kernel(cls_score: np.ndarray, anchors: np.ndarray = None,
           label: np.ndarray = None) -> np.ndarray:
    global LAST_EXEC_NS, LAST_RESULTS
    from concourse.bass_utils import run_bass_kernel_spmd

    cls_score = np.ascontiguousarray(np.asarray(cls_score, dtype=np.float32))
    label = np.ascontiguousarray(np.asarray(label, dtype=np.int32))
    if anchors is None:
        anchors = _default_anchors()
    anchors = np.asarray(anchors, dtype=np.float32)
    assert cls_score.shape == (N_ROIS, M) and label.shape == (N_ROIS,)

    gbb_mean = _host_gbb_mean(cls_score, anchors)

    nc = _get_built()
    in_maps = []
    for i in range(N_CORES):
        sl = slice(i * N_SH, (i + 1) * N_SH)
        in_maps.append({
            "cls_score": np.ascontiguousarray(cls_score[sl]),
            "label": np.ascontiguousarray(label[sl]),
        })

    trace = os.environ.get("KERNEL_TRACE", "0") == "1"
    if trace:
        _ensure_ntff_hook()
    res = run_bass_kernel_spmd(nc, in_maps, core_ids=list(range(N_CORES)),
                               trace=trace)
    LAST_EXEC_NS = res.exec_time_ns
    LAST_RESULTS = res

    outs = np.stack([r["out"] for r in res.results])   # (8, 128, 2)
    d_total = float(outs[:, :, 0].sum(dtype=np.float64))
    n_valid = int(round(float(outs[:, :, 1].sum(dtype=np.float64))))

    loss = (LOSS_WEIGHT * ANCHOR_WEIGHT
            * (d_total / M - gbb_mean * n_valid) / max(n_valid, 1))
    return np.float32(loss)
